# revision 48
# baseline (speedup 1.0000x reference)
# Trainium2 Bass kernel for nn_FFF_v2 (fast-feedforward / MoE tree routing).
#
#   lam   = x @ W.T                      [B, 12] router logits
#   branch= lam > 0                      tree descent decisions
#   node  = (2^i - 1) + sum_{j<i} branch_j 2^(i-1-j)
#   out   = sum_i lam_i * Y[node_i]      [B, 4096]
#
# Sharding: data-parallel on batch across 8 cores (1024 rows each); W and Y
# replicated.  Per core:
#   - router logits via PE matmul in fp16 hi/lo (sign fidelity vs the fp32
#     reference needs ~1e-5 logits; fp16 hi/lo reproduces that at 1 cyc/row).
#     W_h and W_l ride in ONE 24-wide stationary so x_h streams once
#     (64 chunk matmuls per macro instead of 96).
#   - tree-node ids via small f32r matmuls (exact for integers < 2^12)
#   - levels 0..8: scaled-one-hot bf16 matmul against SBUF-resident shallow
#     Y rows (plain contiguous DMA load -- each 128-node chunk is a
#     contiguous block of Y, no gather needed)
#   - levels 9..11: dma_gather of bf16 Y rows from HBM + a fused
#     scalar_tensor_tensor FMA chain on the vector engine
#   - index replication for the 8 Q7 descriptor-gen cores via a PE
#     replication matmul (16 -> 128 partitions), no DRAM bounce
#
# Schedule notes (from NTFF profiling of the previous version):
#   - DMA is the bottleneck (~55 MB/core, ~400 GB/s peak). Startup order
#     matters: xt macro 0 is issued FIRST on the sync+scalar queues, shallow
#     Y right after on the same queues, consts on the gpsimd queue, so the
#     router starts at ~17 us instead of ~43 us.
#   - gather index computation for both subtiles is hoisted ahead of the
#     one-hot matmuls so the swdge queues stay fed.
#   - output writes go on the vector engine's DMA queue (otherwise idle).
import numpy as np

DEPTH = 12
B = 8192
D = 4096
N_NODES = 4095
NCORES = 8
B_LOC = B // NCORES          # 1024 rows per core

MACRO = 256                  # batch rows per macro tile
SUB = 128                    # rows per subtile (one partition block)
NSUB = MACRO // SUB          # 2
NMACRO = B_LOC // MACRO      # 4

K_MM = 9                     # levels 0..K_MM-1 handled by one-hot matmul
N_SH = 2 ** K_MM - 1         # shallow nodes (511)
NCHUNK = (N_SH + 127) // 128  # 4
N_GL = DEPTH - K_MM          # gather levels (3)

_CACHE = {}


def _level_of(n):
    # level i spans nodes [2^i - 1, 2^(i+1) - 1)
    lev = 0
    while n >= 2 ** (lev + 1) - 1:
        lev += 1
    return lev


def _host_consts():
    # powT[j, i] = powmat[i, j] = 2^(i-1-j) for j < i  (lhsT of prefix matmul)
    powT = np.zeros((DEPTH, DEPTH), np.float32)
    for i in range(DEPTH):
        for j in range(i):
            powT[j, i] = float(1 << (i - 1 - j))
    # offsd[p, l*8+f] = 2^(K_MM+l) - 1 for deep levels, on all 128 partitions
    offsd = np.zeros((128, N_GL * (SUB // 16)), np.float32)
    for l in range(N_GL):
        offsd[:, l * (SUB // 16) : (l + 1) * (SUB // 16)] = float(
            (1 << (K_MM + l)) - 1
        )
    # bselT[l, c*128+p] = 1 if level(c*128+p) == l else 0   (lhsT of bc matmul)
    bselT = np.zeros((DEPTH, NCHUNK * 128), np.float32)
    # nrel[p, c] = node - (2^level - 1), or -1 for pad positions
    nrel = np.full((128, NCHUNK), -1.0, np.float32)
    for c in range(NCHUNK):
        for p in range(128):
            n = c * 128 + p
            if n < N_SH:
                lev = _level_of(n)
                bselT[lev, c * 128 + p] = 1.0
                nrel[p, c] = float(n - ((1 << lev) - 1))
    # identity for PE transposes of lam/prefix tiles
    ident = np.eye(2 * DEPTH, dtype=np.float32)
    # replT[k, m] = 1 if k == m % 16: PE matmul that replicates a
    # 16-partition tile across all 8 groups of 16 partitions
    replT = np.tile(np.eye(16, dtype=np.float32), (1, 8))
    return powT, offsd, bselT, nrel, ident, replT


def _build_program():
    import concourse.bass as bass
    import concourse.bacc as bacc
    import concourse.mybir as mybir
    import concourse.tile as tile
    from contextlib import ExitStack

    dt = mybir.dt
    f32 = dt.float32
    f32r = dt.float32r
    bf16 = dt.bfloat16
    i16 = dt.int16
    f16 = dt.float16
    Alu = mybir.AluOpType

    nc = bacc.Bacc(trn_type="TRN2", num_swdge_queues=4)

    ydt = bf16  # Y payload: bf16 halves gather bytes + 4x one-hot matmul rate
    # x^T fp16 hi + fp16 lo halves.  (An fp8 x_l was tried and is NOT safe:
    # the reference's branch signs come from ITS f32 matmul, whose own
    # accumulation noise is ~2e-4 -- the kernel's logits must match far
    # below that, which needs the full fp16 residual.)
    xt_d = nc.dram_tensor("xt", [NMACRO, 128, 32, MACRO], f16, kind="ExternalInput")
    xl_d = nc.dram_tensor("xl", [NMACRO, 128, 32, MACRO], f16, kind="ExternalInput")
    y_d = nc.dram_tensor("y", [N_NODES, D], ydt, kind="ExternalInput")
    # wt[p, c, 0:12] = W_h.T chunk, wt[p, c, 32:44] = W_l.T chunk (zeros
    # between: engine PSUM reads must start at a 32-aligned partition, so
    # the W_l rows of the router PSUM tile live at partition 32)
    WTW = 32 + DEPTH
    wt_d = nc.dram_tensor("wt", [128, 32, WTW], f16, kind="ExternalInput")
    powt_d = nc.dram_tensor("powt", [DEPTH, DEPTH], f32r, kind="ExternalInput")
    offsd_d = nc.dram_tensor(
        "offsd", [128, N_GL * (SUB // 16)], f32, kind="ExternalInput"
    )
    bselt_d = nc.dram_tensor("bselt", [DEPTH, NCHUNK * 128], f32r, kind="ExternalInput")
    nrel_d = nc.dram_tensor("nrel", [128, NCHUNK], f32, kind="ExternalInput")
    ident_d = nc.dram_tensor("ident", [2 * DEPTH, 2 * DEPTH], f32r, kind="ExternalInput")
    replt_d = nc.dram_tensor("replt", [16, 128], f32r, kind="ExternalInput")
    out_d = nc.dram_tensor("out", [B_LOC, D], bf16, kind="ExternalOutput")

    with tile.TileContext(nc) as tc, ExitStack() as ctx:
        consts = ctx.enter_context(tc.tile_pool(name="consts", bufs=1))
        xt_p = ctx.enter_context(tc.tile_pool(name="xt", bufs=2))
        small = ctx.enter_context(tc.tile_pool(name="small", bufs=2))
        small4 = ctx.enter_context(tc.tile_pool(name="small4", bufs=6))
        st_p = ctx.enter_context(tc.tile_pool(name="st", bufs=6))
        g_p = ctx.enter_context(tc.tile_pool(name="g", bufs=6))
        acc_p = ctx.enter_context(tc.tile_pool(name="acc", bufs=2))
        scr_p = ctx.enter_context(tc.tile_pool(name="scr", bufs=1))
        out_p = ctx.enter_context(tc.tile_pool(name="outp", bufs=2))
        ps_a = ctx.enter_context(tc.tile_pool(name="psa", bufs=1, space="PSUM"))
        ps_bc = ctx.enter_context(tc.tile_pool(name="psbc", bufs=2, space="PSUM"))
        ps_tp = ctx.enter_context(tc.tile_pool(name="pstp", bufs=2, space="PSUM"))
        ps_out = ctx.enter_context(tc.tile_pool(name="psout", bufs=2, space="PSUM"))

        # ---- startup-critical DMAs in priority order.  Queue order on each
        # engine queue is program order, so: router weights, then x macro 0
        # (hi on sync queue, lo on scalar queue), then the shallow Y rows
        # split across both queues.  Everything else (consts) goes on the
        # gpsimd engine queue so it cannot delay these. ----
        wt_sb = consts.tile([128, 32, WTW], f16)
        nc.sync.dma_start(wt_sb[:], wt_d.ap())

        xt0 = xt_p.tile([128, 32, MACRO], f16, tag="xt")
        nc.sync.dma_start(xt0[:], xt_d.ap()[0])
        xl0 = xt_p.tile([128, 32, MACRO], f16, tag="xl")
        nc.scalar.dma_start(xl0[:], xl_d.ap()[0])

        # shallow Y: chunk c is the contiguous block Y[c*128:(c+1)*128], one
        # row per partition -- a plain strided DMA, no gather required.
        # Behind x macro 0 in queue order on both queues.
        ysh = []
        for c in range(NCHUNK):
            yc = consts.tile([128, D], ydt, tag=f"yc{c}")
            lo = c * 128
            eng = nc.sync if c % 2 == 0 else nc.scalar
            eng.dma_start(yc[:], y_d.ap()[lo : lo + 128, :])
            ysh.append(yc)

        powt_sb = consts.tile([DEPTH, DEPTH], f32r)
        nc.gpsimd.dma_start(powt_sb[:], powt_d.ap())
        offsd_sb = consts.tile([128, N_GL * (SUB // 16)], f32)
        nc.gpsimd.dma_start(offsd_sb[:], offsd_d.ap())
        bselt_sb = consts.tile([DEPTH, NCHUNK * 128], f32r)
        nc.gpsimd.dma_start(bselt_sb[:], bselt_d.ap())
        nrel_sb = consts.tile([128, NCHUNK], f32)
        nc.gpsimd.dma_start(nrel_sb[:], nrel_d.ap())
        ident_sb = consts.tile([2 * DEPTH, 2 * DEPTH], f32r)
        nc.gpsimd.dma_start(ident_sb[:], ident_d.ap())
        replt_sb = consts.tile([16, 128], f32r)
        nc.gpsimd.dma_start(replt_sb[:], replt_d.ap())

        def _emit_fma(work):
            # full-width bf16 FMA tree: tensor_scalar (4x mode) products,
            # tensor_tensor (2x mode) adds, accumulating in-place into out_t
            # with a single scratch tile.  (A fused scalar_tensor_tensor
            # chain is NOT faster: STT runs in 1x DVE mode, ~2.5x slower.)
            # Processed in column halves so each half's writeback DMA starts
            # as soon as its chain finishes -- halves the end-of-kernel DVE
            # drain.
            gt, lamb, po16f, row0 = work
            assert N_GL == 3
            t_a = scr_p.tile([SUB, D], bf16, tag="ta")
            out_t = out_p.tile([SUB, D], bf16, tag="out")
            nc.vector.tensor_scalar(
                out_t[:], gt[0][:, 0, :], lamb[:, K_MM : K_MM + 1], None,
                Alu.mult,
            )
            nc.vector.tensor_scalar(
                t_a[:], gt[1][:, 0, :], lamb[:, K_MM + 1 : K_MM + 2], None,
                Alu.mult,
            )
            nc.vector.tensor_tensor(out_t[:], out_t[:], t_a[:], Alu.add)
            nc.vector.tensor_scalar(
                t_a[:], gt[2][:, 0, :], lamb[:, K_MM + 2 : K_MM + 3], None,
                Alu.mult,
            )
            nc.vector.tensor_tensor(out_t[:], out_t[:], t_a[:], Alu.add)
            # final add in halves so the first half's writeback DMA starts
            # one op earlier (cheap overlap; the big ops stay full-width
            # where the DVE's fixed per-op cost amortizes best)
            for h in range(2):
                cs = slice(h * (D // 2), (h + 1) * (D // 2))
                nc.vector.tensor_tensor(
                    out_t[:, cs], out_t[:, cs], po16f[:, cs], Alu.add
                )
                nc.sync.dma_start(
                    out_d.ap()[row0 : row0 + SUB, cs], out_t[:, cs]
                )

        xt = xt0
        xl = xl0
        pend = None
        for m in range(NMACRO):
            # prefetch next x macro on the gpsimd queue: the Pool engine is
            # idle at macro-top (its descriptor-gen work comes later, after
            # idxr), so the issue lands immediately -- on sync it would sit
            # behind out-write issues, on scalar behind the po16f copies,
            # both of which delayed the prefetch by ~10us per macro.
            if m + 1 < NMACRO:
                xt_nxt = xt_p.tile([128, 32, MACRO], f16, tag="xt")
                nc.gpsimd.dma_start(xt_nxt[:], xt_d.ap()[m + 1])
                xl_nxt = xt_p.tile([128, 32, MACRO], f16, tag="xl")
                nc.gpsimd.dma_start(xl_nxt[:], xl_d.ap()[m + 1])

            # ---- router: lam^T [12, MACRO] = W @ x^T via fp16 hi/lo ----
            # pass 1: [W_h|pad|W_l] (44-wide stationary) over x_h ->
            #         rows 0:12 and 32:44 in one x_h stream
            # pass 2: W_h over x_l, accumulated onto rows 0:12
            lam_ps = ps_a.tile([WTW, MACRO], f32, tag="lam24")
            for c in range(32):
                nc.tensor.matmul(
                    lam_ps[:], wt_sb[:, c, :], xt[:, c, :],
                    start=(c == 0), stop=(c == 31),
                )
            for c in range(32):
                nc.tensor.matmul(
                    lam_ps[:DEPTH, :], wt_sb[:, c, :DEPTH], xl[:, c, :],
                    start=False, stop=(c == 31),
                )

            # fold: lam = rows[0:12] + rows[32:44]  (one operand staged to
            # SBUF so the DVE reads only one PSUM operand)
            hl = small.tile([DEPTH, MACRO], f32r, tag="hl")
            nc.scalar.copy(hl[:], lam_ps[32:, :])
            # bcr = [pfx^T | lam^T] joint rhs for the bc matmul
            bcr = small.tile([DEPTH, 2 * MACRO], f32r, tag="bcr")
            nc.vector.tensor_tensor(bcr[:, MACRO:], lam_ps[:DEPTH, :], hl[:], Alu.add)
            branch = small.tile([DEPTH, MACRO], f32r, tag="branch")
            nc.vector.tensor_scalar(branch[:], bcr[:, MACRO:], 0.0, None, Alu.is_gt)

            # prefix^T [12, MACRO] = powmat @ branch  (f32r, exact ints)
            pb_ps = ps_a.tile([DEPTH, MACRO], f32, tag="pb")
            nc.tensor.matmul(pb_ps[:], powt_sb[:], branch[:], start=True, stop=True)
            nc.scalar.copy(bcr[:, :MACRO], pb_ps[:])

            # ---- gather-index path for BOTH subtiles first, so the swdge
            # queues start pulling deep Y rows while the PE does the
            # one-hot matmuls ----
            lambs = []
            for s in range(NSUB):
                bsl = slice(s * SUB, (s + 1) * SUB)
                tpw = ps_tp.tile([SUB, 160], f32, tag="tpw")
                tp_ps = tpw[:, :DEPTH]
                nc.tensor.matmul(
                    tp_ps, bcr[:, MACRO + s * SUB : MACRO + (s + 1) * SUB],
                    ident_sb[:DEPTH, :DEPTH],
                    start=True, stop=True,
                )
                lamb = small4.tile([SUB, DEPTH], f32, tag="lamb")
                nc.vector.tensor_copy(lamb[:], tp_ps)
                lambs.append(lamb)

                # node ids: per-16-column PE transposes of prefix^T
                w_ps = tpw[:16, 16 : 16 + (SUB // 16) * DEPTH].rearrange(
                    "p (f l) -> p f l", f=SUB // 16
                )
                for f in range(SUB // 16):
                    nc.tensor.matmul(
                        w_ps[:, f, :],
                        bcr[:, s * SUB + f * 16 : s * SUB + (f + 1) * 16],
                        ident_sb[:DEPTH, :DEPTH],
                        start=True, stop=True,
                    )
                # deep-level relative ids to SBUF, then replicate to all 8
                # 16-partition groups via the PE (no DRAM bounce)
                idx16f = small4.tile([16, SUB // 16, N_GL], f32r, tag="idx16f")
                nc.scalar.copy(idx16f[:], w_ps[:, :, K_MM:])
                rep_ps = tpw[:, 128 : 128 + (SUB // 16) * N_GL]
                nc.tensor.matmul(
                    rep_ps,
                    replt_sb[:],
                    idx16f[:].rearrange("p f l -> p (f l)"),
                    start=True, stop=True,
                )
                idxr = small4.tile([128, N_GL, SUB // 16], i16, tag="idxr")
                nc.vector.tensor_tensor(
                    idxr[:],
                    rep_ps.rearrange("p (f l) -> p f l", f=SUB // 16).rearrange(
                        "p f l -> p l f"
                    ),
                    offsd_sb[:].rearrange("p (l f) -> p l f", l=N_GL),
                    Alu.add,
                )

                # gather deep levels from HBM (bf16 rows)
                if s == 0:
                    gts = [[], []]
                for li in range(N_GL):
                    g = g_p.tile([128, 1, D], ydt, tag="g")
                    nc.gpsimd.dma_gather(
                        g[:], y_d.ap(), idxr[:, li, :],
                        SUB, SUB, D,
                        queue_num=(3 * (NSUB * m + s) + li) % 4,
                    )
                    gts[s].append(g)

            # ---- S^T build: one chunk of 128 shallow nodes at a time ----
            st = []
            for c in range(NCHUNK):
                bc_ps = ps_bc.tile([128, 2 * MACRO], f32, tag="bc")
                nc.tensor.matmul(
                    bc_ps[:], bselt_sb[:, c * 128 : (c + 1) * 128],
                    bcr[:], start=True, stop=True,
                )
                lbc = small.tile([128, MACRO], f32, tag="lbc")
                nc.scalar.copy(lbc[:], bc_ps[:, MACRO:])
                stc = st_p.tile([128, MACRO], ydt, tag="st")
                nc.vector.scalar_tensor_tensor(
                    stc[:], bc_ps[:, :MACRO], nrel_sb[:, c : c + 1], lbc[:],
                    Alu.is_equal, Alu.mult,
                )
                st.append(stc)

            # deferred FMA from the previous subtile: emitted here so in the
            # DVE's in-order queue it lands AFTER this macro's small ops
            # (fold/branch/idxr/stc) -- otherwise the 18us FMA block delays
            # them, which delays the gathers and the PE's one-hot
            if pend is not None:
                _emit_fma(pend)
                pend = None

            for s in range(NSUB):
                bsl = slice(s * SUB, (s + 1) * SUB)

                # one-hot matmul: shallow-level contribution (bf16 PE), with
                # the PSUM q-tiles downcast to one full-width bf16 row on the
                # scalar engine
                po16f = acc_p.tile([SUB, D], bf16, tag="po16f")
                for q in range(D // 512):
                    po = ps_out.tile([SUB, 512], f32, tag="po")
                    for c in range(NCHUNK):
                        nc.tensor.matmul(
                            po[:], st[c][:, bsl],
                            ysh[c][:, q * 512 : (q + 1) * 512],
                            start=(c == 0), stop=(c == NCHUNK - 1),
                        )
                    nc.scalar.copy(po16f[:, q * 512 : (q + 1) * 512], po[:])

                work = (gts[s], lambs[s], po16f, m * MACRO + s * SUB)
                if s == 0:
                    _emit_fma(work)
                else:
                    pend = work

            if m + 1 < NMACRO:
                xt = xt_nxt
                xl = xl_nxt

        if pend is not None:
            _emit_fma(pend)

    nc.compile()
    return nc


def _patch_walrus_passes():
    # The default walrus pass list in this environment omits
    # lower_custom_kernel, which the Pool custom instructions (dma_gather)
    # need. Inject it in front of codegen.
    import concourse.bass_utils as bu

    if getattr(bu, "_ant_lck_patched", False):
        return
    bu._ant_lck_patched = True
    orig = bu.run_command

    def run_command(argv, **kw):
        if argv and "walrus_driver" in str(argv[0]):
            argv = list(argv)
            for i, a in enumerate(argv):
                if a == "--pass" and "lower_custom_kernel" not in argv[i + 1]:
                    argv[i + 1] = argv[i + 1].replace(
                        "codegen", "lower_custom_kernel,codegen"
                    )
                    break
        return orig(argv, **kw)

    bu.run_command = run_command


def _get_program():
    if "nc" not in _CACHE:
        _CACHE["nc"] = _build_program()
    return _CACHE["nc"]


def _prep_in_maps(x, W, Y):
    import ml_dtypes

    powT, offsd, bselT, nrel, ident, replT = _host_consts()
    Y = np.ascontiguousarray(Y, np.float32).astype(ml_dtypes.bfloat16)
    # fp16 hi/lo split of W packed as [128, 32, 44]: chunk c holds
    # [W_h.T rows | zero pad | W_l.T rows] (W_l at 32-aligned PSUM rows)
    Wf = np.ascontiguousarray(W, np.float32)
    W_h = Wf.astype(np.float16)
    W_l = (Wf - W_h.astype(np.float32)).astype(np.float16)
    w_h = W_h.T.reshape(32, 128, DEPTH)
    w_l = W_l.T.reshape(32, 128, DEPTH)
    pad = np.zeros((32, 128, 32 - DEPTH), np.float16)
    wt = np.ascontiguousarray(
        np.concatenate([w_h, pad, w_l], axis=2).transpose(1, 0, 2), np.float16
    )
    in_maps = []
    xr = x.reshape(NCORES, B_LOC, D)
    for c in range(NCORES):
        xt = np.ascontiguousarray(xr[c].T, np.float32)  # [D, B_LOC]
        x_h = xt.astype(np.float16)
        x_l = (xt - x_h.astype(np.float32)).astype(np.float16)
        xtm = np.ascontiguousarray(
            x_h.reshape(32, 128, NMACRO, MACRO).transpose(2, 1, 0, 3),
            np.float16,
        )
        xlm = np.ascontiguousarray(
            x_l.reshape(32, 128, NMACRO, MACRO).transpose(2, 1, 0, 3),
            np.float16,
        )
        in_maps.append(
            {
                "xt": xtm, "xl": xlm, "y": Y, "wt": wt,
                "powt": powT, "offsd": offsd, "bselt": bselT, "nrel": nrel,
                "ident": ident, "replt": replT,
            }
        )
    return in_maps


def kernel(x, W, Y, _trace=False):
    from concourse.bass_utils import run_bass_kernel_spmd

    _patch_walrus_passes()

    nc = _get_program()
    in_maps = _prep_in_maps(np.asarray(x), np.asarray(W), np.asarray(Y))
    res = run_bass_kernel_spmd(nc, in_maps, list(range(NCORES)), trace=_trace)
    out = np.concatenate(
        [np.asarray(res.results[c]["out"]).astype(np.float32) for c in range(NCORES)],
        axis=0,
    )
    if _trace:
        _CACHE["last_result"] = res
    return out


# revision 49
# speedup vs baseline: 1.0440x; 1.0440x over previous
# Trainium2 Bass kernel for nn_FFF_v2 (fast-feedforward / MoE tree routing).
#
#   lam   = x @ W.T                      [B, 12] router logits
#   branch= lam > 0                      tree descent decisions
#   node  = (2^i - 1) + sum_{j<i} branch_j 2^(i-1-j)
#   out   = sum_i lam_i * Y[node_i]      [B, 4096]
#
# Sharding: data-parallel on batch across 8 cores (1024 rows each); W and Y
# replicated.  Per core:
#   - router logits via PE matmul in fp16 hi/lo (sign fidelity vs the fp32
#     reference needs ~1e-5 logits; fp16 hi/lo reproduces that at 1 cyc/row).
#     W_h and W_l ride in ONE 24-wide stationary so x_h streams once
#     (64 chunk matmuls per macro instead of 96).
#   - tree-node ids via small f32r matmuls (exact for integers < 2^12)
#   - levels 0..8: scaled-one-hot bf16 matmul against SBUF-resident shallow
#     Y rows (plain contiguous DMA load -- each 128-node chunk is a
#     contiguous block of Y, no gather needed)
#   - levels 9..11: dma_gather of bf16 Y rows from HBM + a fused
#     scalar_tensor_tensor FMA chain on the vector engine
#   - index replication for the 8 Q7 descriptor-gen cores via a PE
#     replication matmul (16 -> 128 partitions), no DRAM bounce
#
# Schedule notes (from NTFF profiling of the previous version):
#   - DMA is the bottleneck (~55 MB/core, ~400 GB/s peak). Startup order
#     matters: xt macro 0 is issued FIRST on the sync+scalar queues, shallow
#     Y right after on the same queues, consts on the gpsimd queue, so the
#     router starts at ~17 us instead of ~43 us.
#   - gather index computation for both subtiles is hoisted ahead of the
#     one-hot matmuls so the swdge queues stay fed.
#   - output writes go on the vector engine's DMA queue (otherwise idle).
import numpy as np

DEPTH = 12
B = 8192
D = 4096
N_NODES = 4095
NCORES = 8
B_LOC = B // NCORES          # 1024 rows per core

MACRO = 256                  # batch rows per macro tile
SUB = 128                    # rows per subtile (one partition block)
NSUB = MACRO // SUB          # 2
NMACRO = B_LOC // MACRO      # 4

K_MM = 9                     # levels 0..K_MM-1 handled by one-hot matmul
N_SH = 2 ** K_MM - 1         # shallow nodes (511)
NCHUNK = (N_SH + 127) // 128  # 4
N_GL = DEPTH - K_MM          # gather levels (3)

_CACHE = {}


def _level_of(n):
    # level i spans nodes [2^i - 1, 2^(i+1) - 1)
    lev = 0
    while n >= 2 ** (lev + 1) - 1:
        lev += 1
    return lev


def _host_consts():
    # powT[j, i] = powmat[i, j] = 2^(i-1-j) for j < i  (lhsT of prefix matmul)
    powT = np.zeros((DEPTH, DEPTH), np.float32)
    for i in range(DEPTH):
        for j in range(i):
            powT[j, i] = float(1 << (i - 1 - j))
    # offsd[p, l*8+f] = 2^(K_MM+l) - 1 for deep levels, on all 128 partitions
    offsd = np.zeros((128, N_GL * (SUB // 16)), np.float32)
    for l in range(N_GL):
        offsd[:, l * (SUB // 16) : (l + 1) * (SUB // 16)] = float(
            (1 << (K_MM + l)) - 1
        )
    # bselT[l, c*128+p] = 1 if level(c*128+p) == l else 0   (lhsT of bc matmul)
    bselT = np.zeros((DEPTH, NCHUNK * 128), np.float32)
    # nrel[p, c] = node - (2^level - 1), or -1 for pad positions
    nrel = np.full((128, NCHUNK), -1.0, np.float32)
    for c in range(NCHUNK):
        for p in range(128):
            n = c * 128 + p
            if n < N_SH:
                lev = _level_of(n)
                bselT[lev, c * 128 + p] = 1.0
                nrel[p, c] = float(n - ((1 << lev) - 1))
    # identity for PE transposes of lam/prefix tiles
    ident = np.eye(2 * DEPTH, dtype=np.float32)
    # replT[k, m] = 1 if k == m % 16: PE matmul that replicates a
    # 16-partition tile across all 8 groups of 16 partitions
    replT = np.tile(np.eye(16, dtype=np.float32), (1, 8))
    return powT, offsd, bselT, nrel, ident, replT


def _build_program():
    import concourse.bass as bass
    import concourse.bacc as bacc
    import concourse.mybir as mybir
    import concourse.tile as tile
    from contextlib import ExitStack

    dt = mybir.dt
    f32 = dt.float32
    f32r = dt.float32r
    bf16 = dt.bfloat16
    i16 = dt.int16
    f16 = dt.float16
    Alu = mybir.AluOpType

    nc = bacc.Bacc(trn_type="TRN2", num_swdge_queues=4)

    ydt = bf16  # Y payload: bf16 halves gather bytes + 4x one-hot matmul rate
    # x^T fp16 hi + fp16 lo halves.  (An fp8 x_l was tried and is NOT safe:
    # the reference's branch signs come from ITS f32 matmul, whose own
    # accumulation noise is ~2e-4 -- the kernel's logits must match far
    # below that, which needs the full fp16 residual.)
    xt_d = nc.dram_tensor("xt", [NMACRO, 128, 32, MACRO], f16, kind="ExternalInput")
    xl_d = nc.dram_tensor("xl", [NMACRO, 128, 32, MACRO], f16, kind="ExternalInput")
    y_d = nc.dram_tensor("y", [N_NODES, D], ydt, kind="ExternalInput")
    # wt[p, c, 0:12] = W_h.T chunk, wt[p, c, 32:44] = W_l.T chunk (zeros
    # between: engine PSUM reads must start at a 32-aligned partition, so
    # the W_l rows of the router PSUM tile live at partition 32)
    WTW = 32 + DEPTH
    wt_d = nc.dram_tensor("wt", [128, 32, WTW], f16, kind="ExternalInput")
    powt_d = nc.dram_tensor("powt", [DEPTH, DEPTH], f32r, kind="ExternalInput")
    offsd_d = nc.dram_tensor(
        "offsd", [128, N_GL * (SUB // 16)], f32, kind="ExternalInput"
    )
    bselt_d = nc.dram_tensor("bselt", [DEPTH, NCHUNK * 128], f32r, kind="ExternalInput")
    nrel_d = nc.dram_tensor("nrel", [128, NCHUNK], f32, kind="ExternalInput")
    ident_d = nc.dram_tensor("ident", [2 * DEPTH, 2 * DEPTH], f32r, kind="ExternalInput")
    replt_d = nc.dram_tensor("replt", [16, 128], f32r, kind="ExternalInput")
    out_d = nc.dram_tensor("out", [B_LOC, D], bf16, kind="ExternalOutput")

    with tile.TileContext(nc) as tc, ExitStack() as ctx:
        consts = ctx.enter_context(tc.tile_pool(name="consts", bufs=1))
        xt_p = ctx.enter_context(tc.tile_pool(name="xt", bufs=2))
        small = ctx.enter_context(tc.tile_pool(name="small", bufs=2))
        small4 = ctx.enter_context(tc.tile_pool(name="small4", bufs=6))
        st_p = ctx.enter_context(tc.tile_pool(name="st", bufs=6))
        g_p = ctx.enter_context(tc.tile_pool(name="g", bufs=6))
        acc_p = ctx.enter_context(tc.tile_pool(name="acc", bufs=2))
        scr_p = ctx.enter_context(tc.tile_pool(name="scr", bufs=1))
        out_p = ctx.enter_context(tc.tile_pool(name="outp", bufs=2))
        ps_a = ctx.enter_context(tc.tile_pool(name="psa", bufs=1, space="PSUM"))
        ps_bc = ctx.enter_context(tc.tile_pool(name="psbc", bufs=2, space="PSUM"))
        ps_tp = ctx.enter_context(tc.tile_pool(name="pstp", bufs=2, space="PSUM"))
        ps_out = ctx.enter_context(tc.tile_pool(name="psout", bufs=2, space="PSUM"))

        # ---- startup-critical DMAs in priority order.  Queue order on each
        # engine queue is program order, so: router weights, then x macro 0
        # (hi on sync queue, lo on scalar queue), then the shallow Y rows
        # split across both queues.  Everything else (consts) goes on the
        # gpsimd engine queue so it cannot delay these. ----
        wt_sb = consts.tile([128, 32, WTW], f16)
        nc.sync.dma_start(wt_sb[:], wt_d.ap())

        xt0 = xt_p.tile([128, 32, MACRO], f16, tag="xt")
        nc.sync.dma_start(xt0[:], xt_d.ap()[0])
        xl0 = xt_p.tile([128, 32, MACRO], f16, tag="xl")
        nc.scalar.dma_start(xl0[:], xl_d.ap()[0])

        # shallow Y: chunk c is the contiguous block Y[c*128:(c+1)*128], one
        # row per partition -- a plain strided DMA, no gather required.
        # Behind x macro 0 in queue order on both queues.
        ysh = []
        for c in range(NCHUNK):
            yc = consts.tile([128, D], ydt, tag=f"yc{c}")
            lo = c * 128
            eng = nc.sync if c % 2 == 0 else nc.scalar
            eng.dma_start(yc[:], y_d.ap()[lo : lo + 128, :])
            ysh.append(yc)

        powt_sb = consts.tile([DEPTH, DEPTH], f32r)
        nc.gpsimd.dma_start(powt_sb[:], powt_d.ap())
        offsd_sb = consts.tile([128, N_GL * (SUB // 16)], f32)
        nc.gpsimd.dma_start(offsd_sb[:], offsd_d.ap())
        bselt_sb = consts.tile([DEPTH, NCHUNK * 128], f32r)
        nc.gpsimd.dma_start(bselt_sb[:], bselt_d.ap())
        nrel_sb = consts.tile([128, NCHUNK], f32)
        nc.gpsimd.dma_start(nrel_sb[:], nrel_d.ap())
        ident_sb = consts.tile([2 * DEPTH, 2 * DEPTH], f32r)
        nc.gpsimd.dma_start(ident_sb[:], ident_d.ap())
        replt_sb = consts.tile([16, 128], f32r)
        nc.gpsimd.dma_start(replt_sb[:], replt_d.ap())

        def _emit_fma(work):
            # full-width bf16 FMA tree: tensor_scalar (4x mode) products,
            # tensor_tensor (2x mode) adds, accumulating in-place into out_t
            # with a single scratch tile.  (A fused scalar_tensor_tensor
            # chain is NOT faster: STT runs in 1x DVE mode, ~2.5x slower.)
            # Processed in column halves so each half's writeback DMA starts
            # as soon as its chain finishes -- halves the end-of-kernel DVE
            # drain.
            gt, lamb, po16f, row0 = work
            assert N_GL == 3
            t_a = scr_p.tile([SUB, D], bf16, tag="ta")
            out_t = out_p.tile([SUB, D], bf16, tag="out")
            nc.vector.tensor_scalar(
                out_t[:], gt[0][:, 0, :], lamb[:, K_MM : K_MM + 1], None,
                Alu.mult,
            )
            nc.vector.tensor_scalar(
                t_a[:], gt[1][:, 0, :], lamb[:, K_MM + 1 : K_MM + 2], None,
                Alu.mult,
            )
            nc.vector.tensor_tensor(out_t[:], out_t[:], t_a[:], Alu.add)
            nc.vector.tensor_scalar(
                t_a[:], gt[2][:, 0, :], lamb[:, K_MM + 2 : K_MM + 3], None,
                Alu.mult,
            )
            nc.vector.tensor_tensor(out_t[:], out_t[:], t_a[:], Alu.add)
            # final add in halves so the first half's writeback DMA starts
            # one op earlier (cheap overlap; the big ops stay full-width
            # where the DVE's fixed per-op cost amortizes best)
            for h in range(2):
                cs = slice(h * (D // 2), (h + 1) * (D // 2))
                nc.vector.tensor_tensor(
                    out_t[:, cs], out_t[:, cs], po16f[:, cs], Alu.add
                )
                nc.sync.dma_start(
                    out_d.ap()[row0 : row0 + SUB, cs], out_t[:, cs]
                )

        xt = xt0
        xl = xl0
        pend = None
        for m in range(NMACRO):
            # prefetch next x macro on the scalar queue (measured best: on
            # sync the issues would sit behind out-write issues, and on
            # gpsimd they delay the gather descriptor generation)
            if m + 1 < NMACRO:
                xt_nxt = xt_p.tile([128, 32, MACRO], f16, tag="xt")
                nc.scalar.dma_start(xt_nxt[:], xt_d.ap()[m + 1])
                xl_nxt = xt_p.tile([128, 32, MACRO], f16, tag="xl")
                nc.scalar.dma_start(xl_nxt[:], xl_d.ap()[m + 1])

            # ---- router: lam^T [12, MACRO] = W @ x^T via fp16 hi/lo ----
            # pass 1: [W_h|pad|W_l] (44-wide stationary) over x_h ->
            #         rows 0:12 and 32:44 in one x_h stream
            # pass 2: W_h over x_l, accumulated onto rows 0:12
            lam_ps = ps_a.tile([WTW, MACRO], f32, tag="lam24")
            for c in range(32):
                nc.tensor.matmul(
                    lam_ps[:], wt_sb[:, c, :], xt[:, c, :],
                    start=(c == 0), stop=(c == 31),
                )
            for c in range(32):
                nc.tensor.matmul(
                    lam_ps[:DEPTH, :], wt_sb[:, c, :DEPTH], xl[:, c, :],
                    start=False, stop=(c == 31),
                )

            # fold: lam = rows[0:12] + rows[32:44]  (one operand staged to
            # SBUF so the DVE reads only one PSUM operand)
            hl = small.tile([DEPTH, MACRO], f32r, tag="hl")
            nc.scalar.copy(hl[:], lam_ps[32:, :])
            # bcr = [pfx^T | lam^T] joint rhs for the bc matmul
            bcr = small.tile([DEPTH, 2 * MACRO], f32r, tag="bcr")
            nc.vector.tensor_tensor(bcr[:, MACRO:], lam_ps[:DEPTH, :], hl[:], Alu.add)
            branch = small.tile([DEPTH, MACRO], f32r, tag="branch")
            nc.vector.tensor_scalar(branch[:], bcr[:, MACRO:], 0.0, None, Alu.is_gt)

            # prefix^T [12, MACRO] = powmat @ branch  (f32r, exact ints)
            pb_ps = ps_a.tile([DEPTH, MACRO], f32, tag="pb")
            nc.tensor.matmul(pb_ps[:], powt_sb[:], branch[:], start=True, stop=True)
            nc.scalar.copy(bcr[:, :MACRO], pb_ps[:])

            # ---- gather-index path for BOTH subtiles first, so the swdge
            # queues start pulling deep Y rows while the PE does the
            # one-hot matmuls ----
            lambs = []
            for s in range(NSUB):
                bsl = slice(s * SUB, (s + 1) * SUB)
                tpw = ps_tp.tile([SUB, 160], f32, tag="tpw")
                tp_ps = tpw[:, :DEPTH]
                nc.tensor.matmul(
                    tp_ps, bcr[:, MACRO + s * SUB : MACRO + (s + 1) * SUB],
                    ident_sb[:DEPTH, :DEPTH],
                    start=True, stop=True,
                )
                lamb = small4.tile([SUB, DEPTH], f32, tag="lamb")
                nc.vector.tensor_copy(lamb[:], tp_ps)
                lambs.append(lamb)

                # node ids: per-16-column PE transposes of prefix^T
                w_ps = tpw[:16, 16 : 16 + (SUB // 16) * DEPTH].rearrange(
                    "p (f l) -> p f l", f=SUB // 16
                )
                for f in range(SUB // 16):
                    nc.tensor.matmul(
                        w_ps[:, f, :],
                        bcr[:, s * SUB + f * 16 : s * SUB + (f + 1) * 16],
                        ident_sb[:DEPTH, :DEPTH],
                        start=True, stop=True,
                    )
                # deep-level relative ids to SBUF, then replicate to all 8
                # 16-partition groups via the PE (no DRAM bounce)
                idx16f = small4.tile([16, SUB // 16, N_GL], f32r, tag="idx16f")
                nc.scalar.copy(idx16f[:], w_ps[:, :, K_MM:])
                rep_ps = tpw[:, 128 : 128 + (SUB // 16) * N_GL]
                nc.tensor.matmul(
                    rep_ps,
                    replt_sb[:],
                    idx16f[:].rearrange("p f l -> p (f l)"),
                    start=True, stop=True,
                )
                idxr = small4.tile([128, N_GL, SUB // 16], i16, tag="idxr")
                nc.vector.tensor_tensor(
                    idxr[:],
                    rep_ps.rearrange("p (f l) -> p f l", f=SUB // 16).rearrange(
                        "p f l -> p l f"
                    ),
                    offsd_sb[:].rearrange("p (l f) -> p l f", l=N_GL),
                    Alu.add,
                )

                # gather deep levels from HBM (bf16 rows)
                if s == 0:
                    gts = [[], []]
                for li in range(N_GL):
                    g = g_p.tile([128, 1, D], ydt, tag="g")
                    nc.gpsimd.dma_gather(
                        g[:], y_d.ap(), idxr[:, li, :],
                        SUB, SUB, D,
                        queue_num=(3 * (NSUB * m + s) + li) % 4,
                    )
                    gts[s].append(g)

            # ---- S^T build: one chunk of 128 shallow nodes at a time ----
            st = []
            for c in range(NCHUNK):
                bc_ps = ps_bc.tile([128, 2 * MACRO], f32, tag="bc")
                nc.tensor.matmul(
                    bc_ps[:], bselt_sb[:, c * 128 : (c + 1) * 128],
                    bcr[:], start=True, stop=True,
                )
                lbc = small.tile([128, MACRO], f32, tag="lbc")
                nc.scalar.copy(lbc[:], bc_ps[:, MACRO:])
                stc = st_p.tile([128, MACRO], ydt, tag="st")
                nc.vector.scalar_tensor_tensor(
                    stc[:], bc_ps[:, :MACRO], nrel_sb[:, c : c + 1], lbc[:],
                    Alu.is_equal, Alu.mult,
                )
                st.append(stc)

            # deferred FMA from the previous subtile: emitted here so in the
            # DVE's in-order queue it lands AFTER this macro's small ops
            # (fold/branch/idxr/stc) -- otherwise the 18us FMA block delays
            # them, which delays the gathers and the PE's one-hot
            if pend is not None:
                _emit_fma(pend)
                pend = None

            for s in range(NSUB):
                bsl = slice(s * SUB, (s + 1) * SUB)

                # one-hot matmul: shallow-level contribution (bf16 PE), with
                # the PSUM q-tiles downcast to one full-width bf16 row on the
                # scalar engine
                po16f = acc_p.tile([SUB, D], bf16, tag="po16f")
                for q in range(D // 512):
                    po = ps_out.tile([SUB, 512], f32, tag="po")
                    for c in range(NCHUNK):
                        nc.tensor.matmul(
                            po[:], st[c][:, bsl],
                            ysh[c][:, q * 512 : (q + 1) * 512],
                            start=(c == 0), stop=(c == NCHUNK - 1),
                        )
                    nc.scalar.copy(po16f[:, q * 512 : (q + 1) * 512], po[:])

                work = (gts[s], lambs[s], po16f, m * MACRO + s * SUB)
                if s == 0:
                    _emit_fma(work)
                else:
                    pend = work

            if m + 1 < NMACRO:
                xt = xt_nxt
                xl = xl_nxt

        if pend is not None:
            _emit_fma(pend)

    nc.compile()
    return nc


def _patch_walrus_passes():
    # The default walrus pass list in this environment omits
    # lower_custom_kernel, which the Pool custom instructions (dma_gather)
    # need. Inject it in front of codegen.
    import concourse.bass_utils as bu

    if getattr(bu, "_ant_lck_patched", False):
        return
    bu._ant_lck_patched = True
    orig = bu.run_command

    def run_command(argv, **kw):
        if argv and "walrus_driver" in str(argv[0]):
            argv = list(argv)
            for i, a in enumerate(argv):
                if a == "--pass" and "lower_custom_kernel" not in argv[i + 1]:
                    argv[i + 1] = argv[i + 1].replace(
                        "codegen", "lower_custom_kernel,codegen"
                    )
                    break
        return orig(argv, **kw)

    bu.run_command = run_command


def _get_program():
    if "nc" not in _CACHE:
        _CACHE["nc"] = _build_program()
    return _CACHE["nc"]


def _prep_in_maps(x, W, Y):
    import ml_dtypes

    powT, offsd, bselT, nrel, ident, replT = _host_consts()
    Y = np.ascontiguousarray(Y, np.float32).astype(ml_dtypes.bfloat16)
    # fp16 hi/lo split of W packed as [128, 32, 44]: chunk c holds
    # [W_h.T rows | zero pad | W_l.T rows] (W_l at 32-aligned PSUM rows)
    Wf = np.ascontiguousarray(W, np.float32)
    W_h = Wf.astype(np.float16)
    W_l = (Wf - W_h.astype(np.float32)).astype(np.float16)
    w_h = W_h.T.reshape(32, 128, DEPTH)
    w_l = W_l.T.reshape(32, 128, DEPTH)
    pad = np.zeros((32, 128, 32 - DEPTH), np.float16)
    wt = np.ascontiguousarray(
        np.concatenate([w_h, pad, w_l], axis=2).transpose(1, 0, 2), np.float16
    )
    in_maps = []
    xr = x.reshape(NCORES, B_LOC, D)
    for c in range(NCORES):
        xt = np.ascontiguousarray(xr[c].T, np.float32)  # [D, B_LOC]
        x_h = xt.astype(np.float16)
        x_l = (xt - x_h.astype(np.float32)).astype(np.float16)
        xtm = np.ascontiguousarray(
            x_h.reshape(32, 128, NMACRO, MACRO).transpose(2, 1, 0, 3),
            np.float16,
        )
        xlm = np.ascontiguousarray(
            x_l.reshape(32, 128, NMACRO, MACRO).transpose(2, 1, 0, 3),
            np.float16,
        )
        in_maps.append(
            {
                "xt": xtm, "xl": xlm, "y": Y, "wt": wt,
                "powt": powT, "offsd": offsd, "bselt": bselT, "nrel": nrel,
                "ident": ident, "replt": replT,
            }
        )
    return in_maps


def kernel(x, W, Y, _trace=False):
    from concourse.bass_utils import run_bass_kernel_spmd

    _patch_walrus_passes()

    nc = _get_program()
    in_maps = _prep_in_maps(np.asarray(x), np.asarray(W), np.asarray(Y))
    res = run_bass_kernel_spmd(nc, in_maps, list(range(NCORES)), trace=_trace)
    out = np.concatenate(
        [np.asarray(res.results[c]["out"]).astype(np.float32) for c in range(NCORES)],
        axis=0,
    )
    if _trace:
        _CACHE["last_result"] = res
    return out


# revision 50
# speedup vs baseline: 1.0795x; 1.0340x over previous
# Trainium2 Bass kernel for nn_FFF_v2 (fast-feedforward / MoE tree routing).
#
#   lam   = x @ W.T                      [B, 12] router logits
#   branch= lam > 0                      tree descent decisions
#   node  = (2^i - 1) + sum_{j<i} branch_j 2^(i-1-j)
#   out   = sum_i lam_i * Y[node_i]      [B, 4096]
#
# Sharding: data-parallel on batch across 8 cores (1024 rows each); W and Y
# replicated.  Per core:
#   - router logits via PE matmul in fp16 hi/lo (sign fidelity vs the fp32
#     reference needs ~1e-5 logits; fp16 hi/lo reproduces that at 1 cyc/row).
#     W_h and W_l ride in ONE 24-wide stationary so x_h streams once
#     (64 chunk matmuls per macro instead of 96).
#   - tree-node ids via small f32r matmuls (exact for integers < 2^12)
#   - levels 0..8: scaled-one-hot bf16 matmul against SBUF-resident shallow
#     Y rows (plain contiguous DMA load -- each 128-node chunk is a
#     contiguous block of Y, no gather needed)
#   - levels 9..11: dma_gather of bf16 Y rows from HBM + a fused
#     scalar_tensor_tensor FMA chain on the vector engine
#   - index replication for the 8 Q7 descriptor-gen cores via a PE
#     replication matmul (16 -> 128 partitions), no DRAM bounce
#
# Schedule notes (from NTFF profiling of the previous version):
#   - DMA is the bottleneck (~55 MB/core, ~400 GB/s peak). Startup order
#     matters: xt macro 0 is issued FIRST on the sync+scalar queues, shallow
#     Y right after on the same queues, consts on the gpsimd queue, so the
#     router starts at ~17 us instead of ~43 us.
#   - gather index computation for both subtiles is hoisted ahead of the
#     one-hot matmuls so the swdge queues stay fed.
#   - output writes go on the vector engine's DMA queue (otherwise idle).
import numpy as np

DEPTH = 12
B = 8192
D = 4096
N_NODES = 4095
NCORES = 8
B_LOC = B // NCORES          # 1024 rows per core

MACRO = 256                  # batch rows per macro tile
SUB = 128                    # rows per subtile (one partition block)
NSUB = MACRO // SUB          # 2
NMACRO = B_LOC // MACRO      # 4

K_MM = 9                     # levels 0..K_MM-1 handled by one-hot matmul
N_SH = 2 ** K_MM - 1         # shallow nodes (511)
NCHUNK = (N_SH + 127) // 128  # 4
N_GL = DEPTH - K_MM          # gather levels (3)

_CACHE = {}


def _level_of(n):
    # level i spans nodes [2^i - 1, 2^(i+1) - 1)
    lev = 0
    while n >= 2 ** (lev + 1) - 1:
        lev += 1
    return lev


def _host_consts():
    # powT[j, i] = powmat[i, j] = 2^(i-1-j) for j < i  (lhsT of prefix matmul)
    powT = np.zeros((DEPTH, DEPTH), np.float32)
    for i in range(DEPTH):
        for j in range(i):
            powT[j, i] = float(1 << (i - 1 - j))
    # offsd[p, l*8+f] = 2^(K_MM+l) - 1 for deep levels, on all 128 partitions
    offsd = np.zeros((128, N_GL * (SUB // 16)), np.float32)
    for l in range(N_GL):
        offsd[:, l * (SUB // 16) : (l + 1) * (SUB // 16)] = float(
            (1 << (K_MM + l)) - 1
        )
    # bselT[l, c*128+p] = 1 if level(c*128+p) == l else 0   (lhsT of bc matmul)
    bselT = np.zeros((DEPTH, NCHUNK * 128), np.float32)
    # nrel[p, c] = node - (2^level - 1), or -1 for pad positions
    nrel = np.full((128, NCHUNK), -1.0, np.float32)
    for c in range(NCHUNK):
        for p in range(128):
            n = c * 128 + p
            if n < N_SH:
                lev = _level_of(n)
                bselT[lev, c * 128 + p] = 1.0
                nrel[p, c] = float(n - ((1 << lev) - 1))
    # identity for PE transposes of lam/prefix tiles
    ident = np.eye(2 * DEPTH, dtype=np.float32)
    # replT[k, m] = 1 if k == m % 16: PE matmul that replicates a
    # 16-partition tile across all 8 groups of 16 partitions
    replT = np.tile(np.eye(16, dtype=np.float32), (1, 8))
    return powT, offsd, bselT, nrel, ident, replT


def _build_program():
    import concourse.bass as bass
    import concourse.bacc as bacc
    import concourse.mybir as mybir
    import concourse.tile as tile
    from contextlib import ExitStack

    dt = mybir.dt
    f32 = dt.float32
    f32r = dt.float32r
    bf16 = dt.bfloat16
    i16 = dt.int16
    f16 = dt.float16
    Alu = mybir.AluOpType

    nc = bacc.Bacc(trn_type="TRN2", num_swdge_queues=4)

    ydt = bf16  # Y payload: bf16 halves gather bytes + 4x one-hot matmul rate
    # x^T fp16 hi + fp16 lo halves.  (An fp8 x_l was tried and is NOT safe:
    # the reference's branch signs come from ITS f32 matmul, whose own
    # accumulation noise is ~2e-4 -- the kernel's logits must match far
    # below that, which needs the full fp16 residual.)
    xt_d = nc.dram_tensor("xt", [NMACRO, 128, 32, MACRO], f16, kind="ExternalInput")
    xl_d = nc.dram_tensor("xl", [NMACRO, 128, 32, MACRO], f16, kind="ExternalInput")
    y_d = nc.dram_tensor("y", [N_NODES, D], ydt, kind="ExternalInput")
    # wt[p, c, 0:12] = W_h.T chunk, wt[p, c, 32:44] = W_l.T chunk (zeros
    # between: engine PSUM reads must start at a 32-aligned partition, so
    # the W_l rows of the router PSUM tile live at partition 32)
    WTW = 32 + DEPTH
    wt_d = nc.dram_tensor("wt", [128, 32, WTW], f16, kind="ExternalInput")
    powt_d = nc.dram_tensor("powt", [DEPTH, DEPTH], f32r, kind="ExternalInput")
    offsd_d = nc.dram_tensor(
        "offsd", [128, N_GL * (SUB // 16)], f32, kind="ExternalInput"
    )
    bselt_d = nc.dram_tensor("bselt", [DEPTH, NCHUNK * 128], f32r, kind="ExternalInput")
    nrel_d = nc.dram_tensor("nrel", [128, NCHUNK], f32, kind="ExternalInput")
    ident_d = nc.dram_tensor("ident", [2 * DEPTH, 2 * DEPTH], f32r, kind="ExternalInput")
    replt_d = nc.dram_tensor("replt", [16, 128], f32r, kind="ExternalInput")
    out_d = nc.dram_tensor("out", [B_LOC, D], bf16, kind="ExternalOutput")

    with tile.TileContext(nc) as tc, ExitStack() as ctx:
        consts = ctx.enter_context(tc.tile_pool(name="consts", bufs=1))
        xt_p = ctx.enter_context(tc.tile_pool(name="xt", bufs=2))
        small = ctx.enter_context(tc.tile_pool(name="small", bufs=2))
        small4 = ctx.enter_context(tc.tile_pool(name="small4", bufs=6))
        st_p = ctx.enter_context(tc.tile_pool(name="st", bufs=6))
        g_p = ctx.enter_context(tc.tile_pool(name="g", bufs=6))
        acc_p = ctx.enter_context(tc.tile_pool(name="acc", bufs=2))
        scr_p = ctx.enter_context(tc.tile_pool(name="scr", bufs=1))
        out_p = ctx.enter_context(tc.tile_pool(name="outp", bufs=2))
        ps_a = ctx.enter_context(tc.tile_pool(name="psa", bufs=1, space="PSUM"))
        ps_bc = ctx.enter_context(tc.tile_pool(name="psbc", bufs=2, space="PSUM"))
        ps_tp = ctx.enter_context(tc.tile_pool(name="pstp", bufs=2, space="PSUM"))
        ps_out = ctx.enter_context(tc.tile_pool(name="psout", bufs=2, space="PSUM"))

        # ---- startup-critical DMAs in priority order.  Queue order on each
        # engine queue is program order, so: router weights, then x macro 0
        # (hi on sync queue, lo on scalar queue), then the shallow Y rows
        # split across both queues.  Everything else (consts) goes on the
        # gpsimd engine queue so it cannot delay these. ----
        wt_sb = consts.tile([128, 32, WTW], f16)
        nc.sync.dma_start(wt_sb[:], wt_d.ap())

        xt0 = xt_p.tile([128, 32, MACRO], f16, tag="xt")
        nc.sync.dma_start(xt0[:], xt_d.ap()[0])
        xl0 = xt_p.tile([128, 32, MACRO], f16, tag="xl")
        nc.scalar.dma_start(xl0[:], xl_d.ap()[0])

        # shallow Y: chunk c is the contiguous block Y[c*128:(c+1)*128], one
        # row per partition -- a plain strided DMA, no gather required.
        # Behind x macro 0 in queue order on both queues.
        ysh = []
        for c in range(NCHUNK):
            yc = consts.tile([128, D], ydt, tag=f"yc{c}")
            lo = c * 128
            eng = nc.sync if c % 2 == 0 else nc.scalar
            eng.dma_start(yc[:], y_d.ap()[lo : lo + 128, :])
            ysh.append(yc)

        powt_sb = consts.tile([DEPTH, DEPTH], f32r)
        nc.gpsimd.dma_start(powt_sb[:], powt_d.ap())
        offsd_sb = consts.tile([128, N_GL * (SUB // 16)], f32)
        nc.gpsimd.dma_start(offsd_sb[:], offsd_d.ap())
        bselt_sb = consts.tile([DEPTH, NCHUNK * 128], f32r)
        nc.gpsimd.dma_start(bselt_sb[:], bselt_d.ap())
        nrel_sb = consts.tile([128, NCHUNK], f32)
        nc.gpsimd.dma_start(nrel_sb[:], nrel_d.ap())
        ident_sb = consts.tile([2 * DEPTH, 2 * DEPTH], f32r)
        nc.gpsimd.dma_start(ident_sb[:], ident_d.ap())
        replt_sb = consts.tile([16, 128], f32r)
        nc.gpsimd.dma_start(replt_sb[:], replt_d.ap())

        def _emit_fma(work):
            # full-width bf16 FMA tree: tensor_scalar (4x mode) products,
            # tensor_tensor (2x mode) adds, accumulating in-place into out_t
            # with a single scratch tile.  (A fused scalar_tensor_tensor
            # chain is NOT faster: STT runs in 1x DVE mode, ~2.5x slower.)
            # Processed in column halves so each half's writeback DMA starts
            # as soon as its chain finishes -- halves the end-of-kernel DVE
            # drain.
            gt, lamb, po16f, row0 = work
            assert N_GL == 3
            t_a = scr_p.tile([SUB, D], bf16, tag="ta")
            out_t = out_p.tile([SUB, D], bf16, tag="out")
            for h in range(2):
                cs = slice(h * (D // 2), (h + 1) * (D // 2))
                nc.vector.tensor_scalar(
                    out_t[:, cs], gt[0][:, 0, cs], lamb[:, K_MM : K_MM + 1],
                    None, Alu.mult,
                )
                nc.vector.tensor_scalar(
                    t_a[:, cs], gt[1][:, 0, cs], lamb[:, K_MM + 1 : K_MM + 2],
                    None, Alu.mult,
                )
                nc.vector.tensor_tensor(out_t[:, cs], out_t[:, cs], t_a[:, cs], Alu.add)
                nc.vector.tensor_scalar(
                    t_a[:, cs], gt[2][:, 0, cs], lamb[:, K_MM + 2 : K_MM + 3],
                    None, Alu.mult,
                )
                nc.vector.tensor_tensor(out_t[:, cs], out_t[:, cs], t_a[:, cs], Alu.add)
                nc.vector.tensor_tensor(out_t[:, cs], out_t[:, cs], po16f[:, cs], Alu.add)
                nc.sync.dma_start(
                    out_d.ap()[row0 : row0 + SUB, cs], out_t[:, cs]
                )

        xt = xt0
        xl = xl0
        pend = None
        for m in range(NMACRO):
            # prefetch next x macro on the scalar queue (measured best: on
            # sync the issues would sit behind out-write issues, and on
            # gpsimd they delay the gather descriptor generation)
            if m + 1 < NMACRO:
                xt_nxt = xt_p.tile([128, 32, MACRO], f16, tag="xt")
                nc.scalar.dma_start(xt_nxt[:], xt_d.ap()[m + 1])
                xl_nxt = xt_p.tile([128, 32, MACRO], f16, tag="xl")
                nc.scalar.dma_start(xl_nxt[:], xl_d.ap()[m + 1])

            # ---- router: lam^T [12, MACRO] = W @ x^T via fp16 hi/lo ----
            # pass 1: [W_h|pad|W_l] (44-wide stationary) over x_h ->
            #         rows 0:12 and 32:44 in one x_h stream
            # pass 2: W_h over x_l, accumulated onto rows 0:12
            lam_ps = ps_a.tile([WTW, MACRO], f32, tag="lam24")
            for c in range(32):
                nc.tensor.matmul(
                    lam_ps[:], wt_sb[:, c, :], xt[:, c, :],
                    start=(c == 0), stop=(c == 31),
                )
            for c in range(32):
                nc.tensor.matmul(
                    lam_ps[:DEPTH, :], wt_sb[:, c, :DEPTH], xl[:, c, :],
                    start=False, stop=(c == 31),
                )

            # fold: lam = rows[0:12] + rows[32:44]  (one operand staged to
            # SBUF so the DVE reads only one PSUM operand)
            hl = small.tile([DEPTH, MACRO], f32r, tag="hl")
            nc.scalar.copy(hl[:], lam_ps[32:, :])
            # bcr = [pfx^T | lam^T] joint rhs for the bc matmul
            bcr = small.tile([DEPTH, 2 * MACRO], f32r, tag="bcr")
            nc.vector.tensor_tensor(bcr[:, MACRO:], lam_ps[:DEPTH, :], hl[:], Alu.add)
            branch = small.tile([DEPTH, MACRO], f32r, tag="branch")
            nc.vector.tensor_scalar(branch[:], bcr[:, MACRO:], 0.0, None, Alu.is_gt)

            # prefix^T [12, MACRO] = powmat @ branch  (f32r, exact ints)
            pb_ps = ps_a.tile([DEPTH, MACRO], f32, tag="pb")
            nc.tensor.matmul(pb_ps[:], powt_sb[:], branch[:], start=True, stop=True)
            nc.scalar.copy(bcr[:, :MACRO], pb_ps[:])

            # ---- gather-index path for BOTH subtiles first, so the swdge
            # queues start pulling deep Y rows while the PE does the
            # one-hot matmuls ----
            lambs = []
            for s in range(NSUB):
                bsl = slice(s * SUB, (s + 1) * SUB)
                tpw = ps_tp.tile([SUB, 160], f32, tag="tpw")
                tp_ps = tpw[:, :DEPTH]
                nc.tensor.matmul(
                    tp_ps, bcr[:, MACRO + s * SUB : MACRO + (s + 1) * SUB],
                    ident_sb[:DEPTH, :DEPTH],
                    start=True, stop=True,
                )
                lamb = small4.tile([SUB, DEPTH], f32, tag="lamb")
                nc.vector.tensor_copy(lamb[:], tp_ps)
                lambs.append(lamb)

                # node ids: per-16-column PE transposes of prefix^T
                w_ps = tpw[:16, 16 : 16 + (SUB // 16) * DEPTH].rearrange(
                    "p (f l) -> p f l", f=SUB // 16
                )
                for f in range(SUB // 16):
                    nc.tensor.matmul(
                        w_ps[:, f, :],
                        bcr[:, s * SUB + f * 16 : s * SUB + (f + 1) * 16],
                        ident_sb[:DEPTH, :DEPTH],
                        start=True, stop=True,
                    )
                # deep-level relative ids to SBUF, then replicate to all 8
                # 16-partition groups via the PE (no DRAM bounce)
                idx16f = small4.tile([16, SUB // 16, N_GL], f32r, tag="idx16f")
                nc.scalar.copy(idx16f[:], w_ps[:, :, K_MM:])
                rep_ps = tpw[:, 128 : 128 + (SUB // 16) * N_GL]
                nc.tensor.matmul(
                    rep_ps,
                    replt_sb[:],
                    idx16f[:].rearrange("p f l -> p (f l)"),
                    start=True, stop=True,
                )
                idxr = small4.tile([128, N_GL, SUB // 16], i16, tag="idxr")
                nc.vector.tensor_tensor(
                    idxr[:],
                    rep_ps.rearrange("p (f l) -> p f l", f=SUB // 16).rearrange(
                        "p f l -> p l f"
                    ),
                    offsd_sb[:].rearrange("p (l f) -> p l f", l=N_GL),
                    Alu.add,
                )

                # gather deep levels from HBM (bf16 rows)
                if s == 0:
                    gts = [[], []]
                for li in range(N_GL):
                    g = g_p.tile([128, 1, D], ydt, tag="g")
                    nc.gpsimd.dma_gather(
                        g[:], y_d.ap(), idxr[:, li, :],
                        SUB, SUB, D,
                        queue_num=(3 * (NSUB * m + s) + li) % 4,
                    )
                    gts[s].append(g)

            # ---- S^T build: one chunk of 128 shallow nodes at a time ----
            st = []
            for c in range(NCHUNK):
                bc_ps = ps_bc.tile([128, 2 * MACRO], f32, tag="bc")
                nc.tensor.matmul(
                    bc_ps[:], bselt_sb[:, c * 128 : (c + 1) * 128],
                    bcr[:], start=True, stop=True,
                )
                lbc = small.tile([128, MACRO], f32, tag="lbc")
                nc.scalar.copy(lbc[:], bc_ps[:, MACRO:])
                stc = st_p.tile([128, MACRO], ydt, tag="st")
                nc.vector.scalar_tensor_tensor(
                    stc[:], bc_ps[:, :MACRO], nrel_sb[:, c : c + 1], lbc[:],
                    Alu.is_equal, Alu.mult,
                )
                st.append(stc)

            # deferred FMA from the previous subtile: emitted here so in the
            # DVE's in-order queue it lands AFTER this macro's small ops
            # (fold/branch/idxr/stc) -- otherwise the 18us FMA block delays
            # them, which delays the gathers and the PE's one-hot
            if pend is not None:
                _emit_fma(pend)
                pend = None

            for s in range(NSUB):
                bsl = slice(s * SUB, (s + 1) * SUB)

                # one-hot matmul: shallow-level contribution (bf16 PE), with
                # the PSUM q-tiles downcast to one full-width bf16 row on the
                # scalar engine
                po16f = acc_p.tile([SUB, D], bf16, tag="po16f")
                for q in range(D // 512):
                    po = ps_out.tile([SUB, 512], f32, tag="po")
                    for c in range(NCHUNK):
                        nc.tensor.matmul(
                            po[:], st[c][:, bsl],
                            ysh[c][:, q * 512 : (q + 1) * 512],
                            start=(c == 0), stop=(c == NCHUNK - 1),
                        )
                    nc.scalar.copy(po16f[:, q * 512 : (q + 1) * 512], po[:])

                work = (gts[s], lambs[s], po16f, m * MACRO + s * SUB)
                if s == 0:
                    _emit_fma(work)
                else:
                    pend = work

            if m + 1 < NMACRO:
                xt = xt_nxt
                xl = xl_nxt

        if pend is not None:
            _emit_fma(pend)

    nc.compile()
    return nc


def _patch_walrus_passes():
    # The default walrus pass list in this environment omits
    # lower_custom_kernel, which the Pool custom instructions (dma_gather)
    # need. Inject it in front of codegen.
    import concourse.bass_utils as bu

    if getattr(bu, "_ant_lck_patched", False):
        return
    bu._ant_lck_patched = True
    orig = bu.run_command

    def run_command(argv, **kw):
        if argv and "walrus_driver" in str(argv[0]):
            argv = list(argv)
            for i, a in enumerate(argv):
                if a == "--pass" and "lower_custom_kernel" not in argv[i + 1]:
                    argv[i + 1] = argv[i + 1].replace(
                        "codegen", "lower_custom_kernel,codegen"
                    )
                    break
        return orig(argv, **kw)

    bu.run_command = run_command


def _get_program():
    if "nc" not in _CACHE:
        _CACHE["nc"] = _build_program()
    return _CACHE["nc"]


def _prep_in_maps(x, W, Y):
    import ml_dtypes

    powT, offsd, bselT, nrel, ident, replT = _host_consts()
    Y = np.ascontiguousarray(Y, np.float32).astype(ml_dtypes.bfloat16)
    # fp16 hi/lo split of W packed as [128, 32, 44]: chunk c holds
    # [W_h.T rows | zero pad | W_l.T rows] (W_l at 32-aligned PSUM rows)
    Wf = np.ascontiguousarray(W, np.float32)
    W_h = Wf.astype(np.float16)
    W_l = (Wf - W_h.astype(np.float32)).astype(np.float16)
    w_h = W_h.T.reshape(32, 128, DEPTH)
    w_l = W_l.T.reshape(32, 128, DEPTH)
    pad = np.zeros((32, 128, 32 - DEPTH), np.float16)
    wt = np.ascontiguousarray(
        np.concatenate([w_h, pad, w_l], axis=2).transpose(1, 0, 2), np.float16
    )
    in_maps = []
    xr = x.reshape(NCORES, B_LOC, D)
    for c in range(NCORES):
        xt = np.ascontiguousarray(xr[c].T, np.float32)  # [D, B_LOC]
        x_h = xt.astype(np.float16)
        x_l = (xt - x_h.astype(np.float32)).astype(np.float16)
        xtm = np.ascontiguousarray(
            x_h.reshape(32, 128, NMACRO, MACRO).transpose(2, 1, 0, 3),
            np.float16,
        )
        xlm = np.ascontiguousarray(
            x_l.reshape(32, 128, NMACRO, MACRO).transpose(2, 1, 0, 3),
            np.float16,
        )
        in_maps.append(
            {
                "xt": xtm, "xl": xlm, "y": Y, "wt": wt,
                "powt": powT, "offsd": offsd, "bselt": bselT, "nrel": nrel,
                "ident": ident, "replt": replT,
            }
        )
    return in_maps


def kernel(x, W, Y, _trace=False):
    from concourse.bass_utils import run_bass_kernel_spmd

    _patch_walrus_passes()

    nc = _get_program()
    in_maps = _prep_in_maps(np.asarray(x), np.asarray(W), np.asarray(Y))
    res = run_bass_kernel_spmd(nc, in_maps, list(range(NCORES)), trace=_trace)
    out = np.concatenate(
        [np.asarray(res.results[c]["out"]).astype(np.float32) for c in range(NCORES)],
        axis=0,
    )
    if _trace:
        _CACHE["last_result"] = res
    return out


# revision 51
# speedup vs baseline: 1.1233x; 1.0406x over previous
# Trainium2 Bass kernel for nn_FFF_v2 (fast-feedforward / MoE tree routing).
#
#   lam   = x @ W.T                      [B, 12] router logits
#   branch= lam > 0                      tree descent decisions
#   node  = (2^i - 1) + sum_{j<i} branch_j 2^(i-1-j)
#   out   = sum_i lam_i * Y[node_i]      [B, 4096]
#
# Sharding: data-parallel on batch across 8 cores (1024 rows each); W and Y
# replicated.  Per core:
#   - router logits via PE matmul in fp16 hi/lo (sign fidelity vs the fp32
#     reference needs ~1e-5 logits; fp16 hi/lo reproduces that at 1 cyc/row).
#     W_h and W_l ride in ONE 24-wide stationary so x_h streams once
#     (64 chunk matmuls per macro instead of 96).
#   - tree-node ids via small f32r matmuls (exact for integers < 2^12)
#   - levels 0..8: scaled-one-hot bf16 matmul against SBUF-resident shallow
#     Y rows (plain contiguous DMA load -- each 128-node chunk is a
#     contiguous block of Y, no gather needed)
#   - levels 9..11: dma_gather of bf16 Y rows from HBM + a fused
#     scalar_tensor_tensor FMA chain on the vector engine
#   - index replication for the 8 Q7 descriptor-gen cores via a PE
#     replication matmul (16 -> 128 partitions), no DRAM bounce
#
# Schedule notes (from NTFF profiling of the previous version):
#   - DMA is the bottleneck (~55 MB/core, ~400 GB/s peak). Startup order
#     matters: xt macro 0 is issued FIRST on the sync+scalar queues, shallow
#     Y right after on the same queues, consts on the gpsimd queue, so the
#     router starts at ~17 us instead of ~43 us.
#   - gather index computation for both subtiles is hoisted ahead of the
#     one-hot matmuls so the swdge queues stay fed.
#   - output writes go on the vector engine's DMA queue (otherwise idle).
import numpy as np

DEPTH = 12
B = 8192
D = 4096
N_NODES = 4095
NCORES = 8
B_LOC = B // NCORES          # 1024 rows per core

MACRO = 256                  # batch rows per macro tile
SUB = 128                    # rows per subtile (one partition block)
NSUB = MACRO // SUB          # 2
NMACRO = B_LOC // MACRO      # 4

K_MM = 9                     # levels 0..K_MM-1 handled by one-hot matmul
N_SH = 2 ** K_MM - 1         # shallow nodes (511)
NCHUNK = (N_SH + 127) // 128  # 4
N_GL = DEPTH - K_MM          # gather levels (3)

_CACHE = {}


def _level_of(n):
    # level i spans nodes [2^i - 1, 2^(i+1) - 1)
    lev = 0
    while n >= 2 ** (lev + 1) - 1:
        lev += 1
    return lev


def _host_consts():
    # powT[j, i] = powmat[i, j] = 2^(i-1-j) for j < i  (lhsT of prefix matmul)
    powT = np.zeros((DEPTH, DEPTH), np.float32)
    for i in range(DEPTH):
        for j in range(i):
            powT[j, i] = float(1 << (i - 1 - j))
    # offsd[p, l*8+f] = 2^(K_MM+l) - 1 for deep levels, on all 128 partitions
    offsd = np.zeros((128, N_GL * (SUB // 16)), np.float32)
    for l in range(N_GL):
        offsd[:, l * (SUB // 16) : (l + 1) * (SUB // 16)] = float(
            (1 << (K_MM + l)) - 1
        )
    # bselT[l, c*128+p] = 1 if level(c*128+p) == l else 0   (lhsT of bc matmul)
    bselT = np.zeros((DEPTH, NCHUNK * 128), np.float32)
    # nrel[p, c] = node - (2^level - 1), or -1 for pad positions
    nrel = np.full((128, NCHUNK), -1.0, np.float32)
    for c in range(NCHUNK):
        for p in range(128):
            n = c * 128 + p
            if n < N_SH:
                lev = _level_of(n)
                bselT[lev, c * 128 + p] = 1.0
                nrel[p, c] = float(n - ((1 << lev) - 1))
    # identity for PE transposes of lam/prefix tiles
    ident = np.eye(2 * DEPTH, dtype=np.float32)
    # replT[k, m] = 1 if k == m % 16: PE matmul that replicates a
    # 16-partition tile across all 8 groups of 16 partitions
    replT = np.tile(np.eye(16, dtype=np.float32), (1, 8))
    return powT, offsd, bselT, nrel, ident, replT


def _build_program():
    import concourse.bass as bass
    import concourse.bacc as bacc
    import concourse.mybir as mybir
    import concourse.tile as tile
    from contextlib import ExitStack

    dt = mybir.dt
    f32 = dt.float32
    f32r = dt.float32r
    bf16 = dt.bfloat16
    i16 = dt.int16
    f16 = dt.float16
    Alu = mybir.AluOpType

    nc = bacc.Bacc(trn_type="TRN2", num_swdge_queues=4)

    ydt = bf16  # Y payload: bf16 halves gather bytes + 4x one-hot matmul rate
    # x^T fp16 hi + fp16 lo halves.  (An fp8 x_l was tried and is NOT safe:
    # the reference's branch signs come from ITS f32 matmul, whose own
    # accumulation noise is ~2e-4 -- the kernel's logits must match far
    # below that, which needs the full fp16 residual.)
    xt_d = nc.dram_tensor("xt", [NMACRO, 128, 32, MACRO], f16, kind="ExternalInput")
    xl_d = nc.dram_tensor("xl", [NMACRO, 128, 32, MACRO], f16, kind="ExternalInput")
    y_d = nc.dram_tensor("y", [N_NODES, D], ydt, kind="ExternalInput")
    # wt[p, c, 0:12] = W_h.T chunk, wt[p, c, 32:44] = W_l.T chunk (zeros
    # between: engine PSUM reads must start at a 32-aligned partition, so
    # the W_l rows of the router PSUM tile live at partition 32)
    WTW = 32 + DEPTH
    wt_d = nc.dram_tensor("wt", [128, 32, WTW], f16, kind="ExternalInput")
    powt_d = nc.dram_tensor("powt", [DEPTH, DEPTH], f32r, kind="ExternalInput")
    offsd_d = nc.dram_tensor(
        "offsd", [128, N_GL * (SUB // 16)], f32, kind="ExternalInput"
    )
    bselt_d = nc.dram_tensor("bselt", [DEPTH, NCHUNK * 128], f32r, kind="ExternalInput")
    nrel_d = nc.dram_tensor("nrel", [128, NCHUNK], f32, kind="ExternalInput")
    ident_d = nc.dram_tensor("ident", [2 * DEPTH, 2 * DEPTH], f32r, kind="ExternalInput")
    replt_d = nc.dram_tensor("replt", [16, 128], f32r, kind="ExternalInput")
    out_d = nc.dram_tensor("out", [B_LOC, D], bf16, kind="ExternalOutput")

    with tile.TileContext(nc) as tc, ExitStack() as ctx:
        consts = ctx.enter_context(tc.tile_pool(name="consts", bufs=1))
        xt_p = ctx.enter_context(tc.tile_pool(name="xt", bufs=2))
        small = ctx.enter_context(tc.tile_pool(name="small", bufs=2))
        small4 = ctx.enter_context(tc.tile_pool(name="small4", bufs=6))
        st_p = ctx.enter_context(tc.tile_pool(name="st", bufs=6))
        g_p = ctx.enter_context(tc.tile_pool(name="g", bufs=6))
        acc_p = ctx.enter_context(tc.tile_pool(name="acc", bufs=2))
        scr_p = ctx.enter_context(tc.tile_pool(name="scr", bufs=1))
        out_p = ctx.enter_context(tc.tile_pool(name="outp", bufs=2))
        ps_a = ctx.enter_context(tc.tile_pool(name="psa", bufs=1, space="PSUM"))
        ps_bc = ctx.enter_context(tc.tile_pool(name="psbc", bufs=2, space="PSUM"))
        ps_tp = ctx.enter_context(tc.tile_pool(name="pstp", bufs=2, space="PSUM"))
        ps_out = ctx.enter_context(tc.tile_pool(name="psout", bufs=2, space="PSUM"))

        # ---- startup-critical DMAs in priority order.  Queue order on each
        # engine queue is program order, so: router weights, then x macro 0
        # (hi on sync queue, lo on scalar queue), then the shallow Y rows
        # split across both queues.  Everything else (consts) goes on the
        # gpsimd engine queue so it cannot delay these. ----
        wt_sb = consts.tile([128, 32, WTW], f16)
        nc.sync.dma_start(wt_sb[:], wt_d.ap())

        xt0 = xt_p.tile([128, 32, MACRO], f16, tag="xt")
        nc.sync.dma_start(xt0[:], xt_d.ap()[0])
        xl0 = xt_p.tile([128, 32, MACRO], f16, tag="xl")
        nc.scalar.dma_start(xl0[:], xl_d.ap()[0])

        # shallow Y: chunk c is the contiguous block Y[c*128:(c+1)*128], one
        # row per partition -- a plain strided DMA, no gather required.
        # Behind x macro 0 in queue order on both queues.
        ysh = []
        for c in range(NCHUNK):
            yc = consts.tile([128, D], ydt, tag=f"yc{c}")
            lo = c * 128
            eng = nc.sync if c % 2 == 0 else nc.scalar
            eng.dma_start(yc[:], y_d.ap()[lo : lo + 128, :])
            ysh.append(yc)

        powt_sb = consts.tile([DEPTH, DEPTH], f32r)
        nc.gpsimd.dma_start(powt_sb[:], powt_d.ap())
        offsd_sb = consts.tile([128, N_GL * (SUB // 16)], f32)
        nc.gpsimd.dma_start(offsd_sb[:], offsd_d.ap())
        bselt_sb = consts.tile([DEPTH, NCHUNK * 128], f32r)
        nc.gpsimd.dma_start(bselt_sb[:], bselt_d.ap())
        nrel_sb = consts.tile([128, NCHUNK], f32)
        nc.gpsimd.dma_start(nrel_sb[:], nrel_d.ap())
        ident_sb = consts.tile([2 * DEPTH, 2 * DEPTH], f32r)
        nc.gpsimd.dma_start(ident_sb[:], ident_d.ap())
        replt_sb = consts.tile([16, 128], f32r)
        nc.gpsimd.dma_start(replt_sb[:], replt_d.ap())

        def _emit_fma(work):
            # full-width bf16 FMA tree: tensor_scalar (4x mode) products,
            # tensor_tensor (2x mode) adds, accumulating in-place into out_t
            # with a single scratch tile.  (A fused scalar_tensor_tensor
            # chain is NOT faster: STT runs in 1x DVE mode, ~2.5x slower.)
            # Processed in column halves so each half's writeback DMA starts
            # as soon as its chain finishes -- halves the end-of-kernel DVE
            # drain.
            gt, lamb, po16f, row0 = work
            assert N_GL == 3
            t_a = scr_p.tile([SUB, D], bf16, tag="ta")
            out_t = out_p.tile([SUB, D], bf16, tag="out")
            for h in range(2):
                cs = slice(h * (D // 2), (h + 1) * (D // 2))
                nc.vector.tensor_scalar(
                    out_t[:, cs], gt[0][:, 0, cs], lamb[:, K_MM : K_MM + 1],
                    None, Alu.mult,
                )
                nc.vector.tensor_scalar(
                    t_a[:, cs], gt[1][:, 0, cs], lamb[:, K_MM + 1 : K_MM + 2],
                    None, Alu.mult,
                )
                nc.vector.tensor_tensor(out_t[:, cs], out_t[:, cs], t_a[:, cs], Alu.add)
                nc.vector.tensor_scalar(
                    t_a[:, cs], gt[2][:, 0, cs], lamb[:, K_MM + 2 : K_MM + 3],
                    None, Alu.mult,
                )
                nc.vector.tensor_tensor(out_t[:, cs], out_t[:, cs], t_a[:, cs], Alu.add)
                nc.vector.tensor_tensor(out_t[:, cs], out_t[:, cs], po16f[:, cs], Alu.add)
                nc.sync.dma_start(
                    out_d.ap()[row0 : row0 + SUB, cs], out_t[:, cs]
                )

        xt = xt0
        xl = xl0
        pend = None
        for m in range(NMACRO):
            # prefetch next x macro: the hi half on sync -- with the FMA
            # deferred by a subtile, the sync engine's out-issue backlog is
            # drained by macro-top, so this issue lands promptly (on scalar
            # it sat behind the previous macro's 16 po16f copies, stalling
            # the next router ~8us).  The lo half stays on scalar: router
            # pass 2 needs it a few us after pass 1, and this halves each
            # queue's load.
            if m + 1 < NMACRO:
                xt_nxt = xt_p.tile([128, 32, MACRO], f16, tag="xt")
                nc.sync.dma_start(xt_nxt[:], xt_d.ap()[m + 1])
                xl_nxt = xt_p.tile([128, 32, MACRO], f16, tag="xl")
                nc.scalar.dma_start(xl_nxt[:], xl_d.ap()[m + 1])

            # ---- router: lam^T [12, MACRO] = W @ x^T via fp16 hi/lo ----
            # pass 1: [W_h|pad|W_l] (44-wide stationary) over x_h ->
            #         rows 0:12 and 32:44 in one x_h stream
            # pass 2: W_h over x_l, accumulated onto rows 0:12
            lam_ps = ps_a.tile([WTW, MACRO], f32, tag="lam24")
            for c in range(32):
                nc.tensor.matmul(
                    lam_ps[:], wt_sb[:, c, :], xt[:, c, :],
                    start=(c == 0), stop=(c == 31),
                )
            for c in range(32):
                nc.tensor.matmul(
                    lam_ps[:DEPTH, :], wt_sb[:, c, :DEPTH], xl[:, c, :],
                    start=False, stop=(c == 31),
                )

            # fold: lam = rows[0:12] + rows[32:44]  (one operand staged to
            # SBUF so the DVE reads only one PSUM operand)
            hl = small.tile([DEPTH, MACRO], f32r, tag="hl")
            nc.scalar.copy(hl[:], lam_ps[32:, :])
            # bcr = [pfx^T | lam^T] joint rhs for the bc matmul
            bcr = small.tile([DEPTH, 2 * MACRO], f32r, tag="bcr")
            nc.vector.tensor_tensor(bcr[:, MACRO:], lam_ps[:DEPTH, :], hl[:], Alu.add)
            branch = small.tile([DEPTH, MACRO], f32r, tag="branch")
            nc.vector.tensor_scalar(branch[:], bcr[:, MACRO:], 0.0, None, Alu.is_gt)

            # prefix^T [12, MACRO] = powmat @ branch  (f32r, exact ints)
            pb_ps = ps_a.tile([DEPTH, MACRO], f32, tag="pb")
            nc.tensor.matmul(pb_ps[:], powt_sb[:], branch[:], start=True, stop=True)
            nc.scalar.copy(bcr[:, :MACRO], pb_ps[:])

            # ---- gather-index path for BOTH subtiles first, so the swdge
            # queues start pulling deep Y rows while the PE does the
            # one-hot matmuls ----
            lambs = []
            for s in range(NSUB):
                bsl = slice(s * SUB, (s + 1) * SUB)
                tpw = ps_tp.tile([SUB, 160], f32, tag="tpw")
                tp_ps = tpw[:, :DEPTH]
                nc.tensor.matmul(
                    tp_ps, bcr[:, MACRO + s * SUB : MACRO + (s + 1) * SUB],
                    ident_sb[:DEPTH, :DEPTH],
                    start=True, stop=True,
                )
                lamb = small4.tile([SUB, DEPTH], f32, tag="lamb")
                nc.vector.tensor_copy(lamb[:], tp_ps)
                lambs.append(lamb)

                # node ids: per-16-column PE transposes of prefix^T
                w_ps = tpw[:16, 16 : 16 + (SUB // 16) * DEPTH].rearrange(
                    "p (f l) -> p f l", f=SUB // 16
                )
                for f in range(SUB // 16):
                    nc.tensor.matmul(
                        w_ps[:, f, :],
                        bcr[:, s * SUB + f * 16 : s * SUB + (f + 1) * 16],
                        ident_sb[:DEPTH, :DEPTH],
                        start=True, stop=True,
                    )
                # deep-level relative ids to SBUF, then replicate to all 8
                # 16-partition groups via the PE (no DRAM bounce)
                idx16f = small4.tile([16, SUB // 16, N_GL], f32r, tag="idx16f")
                nc.scalar.copy(idx16f[:], w_ps[:, :, K_MM:])
                rep_ps = tpw[:, 128 : 128 + (SUB // 16) * N_GL]
                nc.tensor.matmul(
                    rep_ps,
                    replt_sb[:],
                    idx16f[:].rearrange("p f l -> p (f l)"),
                    start=True, stop=True,
                )
                idxr = small4.tile([128, N_GL, SUB // 16], i16, tag="idxr")
                nc.vector.tensor_tensor(
                    idxr[:],
                    rep_ps.rearrange("p (f l) -> p f l", f=SUB // 16).rearrange(
                        "p f l -> p l f"
                    ),
                    offsd_sb[:].rearrange("p (l f) -> p l f", l=N_GL),
                    Alu.add,
                )

                # gather deep levels from HBM (bf16 rows)
                if s == 0:
                    gts = [[], []]
                for li in range(N_GL):
                    g = g_p.tile([128, 1, D], ydt, tag="g")
                    nc.gpsimd.dma_gather(
                        g[:], y_d.ap(), idxr[:, li, :],
                        SUB, SUB, D,
                        queue_num=(3 * (NSUB * m + s) + li) % 4,
                    )
                    gts[s].append(g)

            # ---- S^T build: one chunk of 128 shallow nodes at a time ----
            st = []
            for c in range(NCHUNK):
                bc_ps = ps_bc.tile([128, 2 * MACRO], f32, tag="bc")
                nc.tensor.matmul(
                    bc_ps[:], bselt_sb[:, c * 128 : (c + 1) * 128],
                    bcr[:], start=True, stop=True,
                )
                lbc = small.tile([128, MACRO], f32, tag="lbc")
                nc.scalar.copy(lbc[:], bc_ps[:, MACRO:])
                stc = st_p.tile([128, MACRO], ydt, tag="st")
                nc.vector.scalar_tensor_tensor(
                    stc[:], bc_ps[:, :MACRO], nrel_sb[:, c : c + 1], lbc[:],
                    Alu.is_equal, Alu.mult,
                )
                st.append(stc)

            # deferred FMA from the previous subtile: emitted here so in the
            # DVE's in-order queue it lands AFTER this macro's small ops
            # (fold/branch/idxr/stc) -- otherwise the 18us FMA block delays
            # them, which delays the gathers and the PE's one-hot
            if pend is not None:
                _emit_fma(pend)
                pend = None

            for s in range(NSUB):
                bsl = slice(s * SUB, (s + 1) * SUB)

                # one-hot matmul: shallow-level contribution (bf16 PE), with
                # the PSUM q-tiles downcast to one full-width bf16 row on the
                # scalar engine
                po16f = acc_p.tile([SUB, D], bf16, tag="po16f")
                for q in range(D // 512):
                    po = ps_out.tile([SUB, 512], f32, tag="po")
                    for c in range(NCHUNK):
                        nc.tensor.matmul(
                            po[:], st[c][:, bsl],
                            ysh[c][:, q * 512 : (q + 1) * 512],
                            start=(c == 0), stop=(c == NCHUNK - 1),
                        )
                    nc.scalar.copy(po16f[:, q * 512 : (q + 1) * 512], po[:])

                work = (gts[s], lambs[s], po16f, m * MACRO + s * SUB)
                if s == 0:
                    _emit_fma(work)
                else:
                    pend = work

            if m + 1 < NMACRO:
                xt = xt_nxt
                xl = xl_nxt

        if pend is not None:
            _emit_fma(pend)

    nc.compile()
    return nc


def _patch_walrus_passes():
    # The default walrus pass list in this environment omits
    # lower_custom_kernel, which the Pool custom instructions (dma_gather)
    # need. Inject it in front of codegen.
    import concourse.bass_utils as bu

    if getattr(bu, "_ant_lck_patched", False):
        return
    bu._ant_lck_patched = True
    orig = bu.run_command

    def run_command(argv, **kw):
        if argv and "walrus_driver" in str(argv[0]):
            argv = list(argv)
            for i, a in enumerate(argv):
                if a == "--pass" and "lower_custom_kernel" not in argv[i + 1]:
                    argv[i + 1] = argv[i + 1].replace(
                        "codegen", "lower_custom_kernel,codegen"
                    )
                    break
        return orig(argv, **kw)

    bu.run_command = run_command


def _get_program():
    if "nc" not in _CACHE:
        _CACHE["nc"] = _build_program()
    return _CACHE["nc"]


def _prep_in_maps(x, W, Y):
    import ml_dtypes

    powT, offsd, bselT, nrel, ident, replT = _host_consts()
    Y = np.ascontiguousarray(Y, np.float32).astype(ml_dtypes.bfloat16)
    # fp16 hi/lo split of W packed as [128, 32, 44]: chunk c holds
    # [W_h.T rows | zero pad | W_l.T rows] (W_l at 32-aligned PSUM rows)
    Wf = np.ascontiguousarray(W, np.float32)
    W_h = Wf.astype(np.float16)
    W_l = (Wf - W_h.astype(np.float32)).astype(np.float16)
    w_h = W_h.T.reshape(32, 128, DEPTH)
    w_l = W_l.T.reshape(32, 128, DEPTH)
    pad = np.zeros((32, 128, 32 - DEPTH), np.float16)
    wt = np.ascontiguousarray(
        np.concatenate([w_h, pad, w_l], axis=2).transpose(1, 0, 2), np.float16
    )
    in_maps = []
    xr = x.reshape(NCORES, B_LOC, D)
    for c in range(NCORES):
        xt = np.ascontiguousarray(xr[c].T, np.float32)  # [D, B_LOC]
        x_h = xt.astype(np.float16)
        x_l = (xt - x_h.astype(np.float32)).astype(np.float16)
        xtm = np.ascontiguousarray(
            x_h.reshape(32, 128, NMACRO, MACRO).transpose(2, 1, 0, 3),
            np.float16,
        )
        xlm = np.ascontiguousarray(
            x_l.reshape(32, 128, NMACRO, MACRO).transpose(2, 1, 0, 3),
            np.float16,
        )
        in_maps.append(
            {
                "xt": xtm, "xl": xlm, "y": Y, "wt": wt,
                "powt": powT, "offsd": offsd, "bselt": bselT, "nrel": nrel,
                "ident": ident, "replt": replT,
            }
        )
    return in_maps


def kernel(x, W, Y, _trace=False):
    from concourse.bass_utils import run_bass_kernel_spmd

    _patch_walrus_passes()

    nc = _get_program()
    in_maps = _prep_in_maps(np.asarray(x), np.asarray(W), np.asarray(Y))
    res = run_bass_kernel_spmd(nc, in_maps, list(range(NCORES)), trace=_trace)
    out = np.concatenate(
        [np.asarray(res.results[c]["out"]).astype(np.float32) for c in range(NCORES)],
        axis=0,
    )
    if _trace:
        _CACHE["last_result"] = res
    return out
